# revision 43
# baseline (speedup 1.0000x reference)
"""Trainium2 Bass kernel for BaichuanAttention (hidden=5120, 40 heads, b=2, s=2048).

Tensor-parallel over heads across 8 NeuronCores, all matmuls in fp16
(full PE rate, fp32 PSUM accumulation):

  Phase A: per-core QKV projection (sharded W_pack rows), output
           feature-major qkvt in DRAM (fp16).
  Phase B: causal attention with transposed-scores formulation:
           S^T[k,q] = K_tile^T @ Q directly gives the P^T layout the PV
           matmul needs -- no per-tile transposes.  exp on the scalar
           engine (constant bias keeps P in fp16 range), row-sums via
           vector adds + gpsimd partition_all_reduce, V loaded
           token-major via DMA transpose.
  Phase C (fused into B's q-chunk loop): AllGather each finished
           attnT chunk across cores, then each core computes o_proj for
           its 640 hidden columns (full 5120-feature contraction) -- no
           ReduceScatter on the critical path; output is column-sharded.
"""

import math
import sys
from collections import deque

for _p in ("/opt/trn_rl_repo",):
    if _p not in sys.path:
        sys.path.insert(0, _p)

import numpy as np

import concourse.bass as bass
import concourse.bass_isa as bass_isa
import concourse.mybir as mybir
import concourse.tile as tile
from concourse import bacc, bass_utils

F16 = mybir.dt.float16
BF16 = mybir.dt.bfloat16
F32 = mybir.dt.float32


class Cfg:
    def __init__(self, hidden=5120, n_heads=40, dh=128, B=2, S=2048, n_cores=8):
        self.hidden = hidden
        self.n_heads = n_heads
        self.dh = dh
        self.B = B
        self.S = S
        self.n_cores = n_cores
        assert dh == 128
        self.HL = n_heads // n_cores          # heads per core (5)
        self.F = 3 * self.HL * dh             # per-core packed qkv rows (1920)
        self.FO = self.HL * dh                # per-core attn feature width (640)
        self.T = B * S                        # total tokens (4096)
        self.KT = hidden // 128               # contraction tiles for qkv (40)
        self.TC = self.T // 512               # token chunks for qkv (8)
        self.QC = S // 512                    # q chunks per batch (4)
        self.SQT = S // 128                   # k tiles per batch (16)
        self.JB = self.FO // 128              # out column blocks per core (5)

    def key(self):
        return (self.hidden, self.n_heads, self.dh, self.B, self.S, self.n_cores)


def build_program(cfg: Cfg, mode: str):
    """mode: 'causal' (causal skip + multiplicative tri masks),
    'dense' (no mask), 'masked' (additive mask input, pre-scaled and
    pre-transposed on host)."""
    assert mode in ("causal", "dense", "masked")
    c = cfg
    nc = bacc.Bacc("TRN2", target_bir_lowering=False, debug=False,
                   num_devices=c.n_cores)
    xt = nc.dram_tensor("xt", [c.hidden, c.T], F16, kind="ExternalInput").ap()
    wqkvt = nc.dram_tensor("wqkvt", [c.hidden, c.F], F16,
                           kind="ExternalInput").ap()
    wot = nc.dram_tensor("wot", [c.hidden, c.FO], F16,
                         kind="ExternalInput").ap()
    mask_ext = None
    if mode == "masked":
        # maskT[k, q] = mask[q, k] * sqrt(dh), fp32
        mask_ext = nc.dram_tensor("maskt", [c.S, c.S], F32,
                                  kind="ExternalInput").ap()
    # column-sharded transposed output: rows = this core's 640 hidden cols
    out_ext = nc.dram_tensor("out", [c.FO, c.T], F16,
                             kind="ExternalOutput").ap()

    inv_sqrt_dh = 1.0 / math.sqrt(c.dh)

    with tile.TileContext(nc) as tc:
        with tc.tile_pool(name="dram", bufs=1, space="DRAM") as dram:
            # q,k features in fp16; v features in bf16 (the softmax P tiles
            # must be bf16 for range, and the PV matmul needs matching dtypes)
            qkt = dram.tile([2 * c.FO, c.T], F16, tag="qkt", name="qkt")
            vt = dram.tile([c.FO, c.T], BF16, tag="vt", name="vt")
            # p-major stage layout: the gathered o_proj input then reads as
            # 5KB-contiguous runs per partition (big DMA descriptors)
            stages = {}
            gaths = {}
            for qc in range(c.QC):
                for b in range(c.B):
                    stages[(qc, b)] = dram.tile(
                        [128, c.HL, 512], F16, tag=f"st{qc}_{b}",
                        name=f"st{qc}_{b}")
                    gaths[(qc, b)] = dram.tile(
                        [c.n_cores, 128, c.HL, 512], F16, tag=f"g{qc}_{b}",
                        name=f"g{qc}_{b}", addr_space="Shared")

            # ---------------- Phase A: QKV projection -------------------
            # qkvt[f, t] = sum_h wqkvt[h, f] * xt[h, t]
            wq_r = wqkvt.rearrange("(ko p) f -> p ko f", p=128)
            xt_r = xt.rearrange("(ko p) t -> p ko t", p=128)
            qkt_r = qkt.rearrange("(ft p) t -> ft p t", p=128)
            vt_r = vt.rearrange("(ft p) t -> ft p t", p=128)
            n_qk_ft = 2 * c.FO // 128  # 10
            splits = [8, 7]
            assert sum(splits) == c.F // 128
            with tc.tile_pool(name="qkv_w", bufs=2) as wpool, \
                 tc.tile_pool(name="qkv_x", bufs=20) as xpool, \
                 tc.tile_pool(name="qkv_o", bufs=8) as opool, \
                 tc.tile_pool(name="qkv_ps", bufs=8, space="PSUM") as pspool:
                ft0 = 0
                for nft in splits:
                    w_sb = wpool.tile([128, c.KT, nft * 128], F16, tag="w")
                    for kq in range(c.KT):
                        nc.sync.dma_start(
                            w_sb[:, kq],
                            wq_r[:, kq, ft0 * 128:(ft0 + nft) * 128])
                    # order 0,4,...: batch-0 and batch-1 early-token chunks
                    # finish first so Phase B can start during A's last pass
                    for tci in (0, 4, 1, 5, 2, 6, 3, 7):
                        pss = [pspool.tile([128, 512], F32, tag="ps",
                                           name=f"ps{i}")
                               for i in range(nft)]
                        for k in range(c.KT):
                            x_sb = xpool.tile([128, 512], F16, tag="x")
                            # alternate queues: 84MB of x needs both HWDGE
                            # queues to stay ahead of the PE
                            eng = nc.scalar if k % 2 == 0 else nc.sync
                            eng.dma_start(
                                x_sb[:], xt_r[:, k, tci * 512:(tci + 1) * 512])
                            for i in range(nft):
                                nc.tensor.matmul(
                                    pss[i][:],
                                    w_sb[:, k, i * 128:(i + 1) * 128],
                                    x_sb[:],
                                    start=(k == 0), stop=(k == c.KT - 1))
                        for i in range(nft):
                            ft = ft0 + i
                            if ft < n_qk_ft:
                                o_sb = opool.tile([128, 512], F16, tag="o")
                                dst = qkt_r[ft, :, tci * 512:(tci + 1) * 512]
                            else:
                                o_sb = opool.tile([128, 512], BF16, tag="ov")
                                dst = vt_r[ft - n_qk_ft, :,
                                           tci * 512:(tci + 1) * 512]
                            nc.vector.tensor_copy(o_sb[:], pss[i][:])
                            nc.scalar.dma_start(dst, o_sb[:])
                    ft0 += nft

            # ------------- Phase B + C: attention + o_proj --------------
            wot_r = wot.rearrange("(fb p) j -> p fb j", p=128)
            with tc.tile_pool(name="att_c", bufs=1) as cpool, \
                 tc.tile_pool(name="att_q", bufs=10) as qpool, \
                 tc.tile_pool(name="att_k", bufs=5) as kpool, \
                 tc.tile_pool(name="att_v", bufs=5) as vpool, \
                 tc.tile_pool(name="att_p", bufs=5) as ppool, \
                 tc.tile_pool(name="att_sm", bufs=2) as smpool, \
                 tc.tile_pool(name="att_o", bufs=2) as aopool, \
                 tc.tile_pool(name="att_ms", bufs=(4 if mode == "masked" else 1)) as mspool, \
                 tc.tile_pool(name="op_w", bufs=1) as wopool, \
                 tc.tile_pool(name="op_g", bufs=2) as gpool, \
                 tc.tile_pool(name="op_o", bufs=2) as copool, \
                 tc.tile_pool(name="ps_s", bufs=3, space="PSUM") as ps_s, \
                 tc.tile_pool(name="ps_pv", bufs=2, space="PSUM") as ps_pv, \
                 tc.tile_pool(name="ps_c", bufs=2, space="PSUM") as ps_c:

                # resident W_o slice: [128, 40, 640] fp16 (~51KB/partition)
                wo_sb = wopool.tile([128, c.KT, c.FO], F16)
                for fb in range(c.KT):
                    nc.sync.dma_start(wo_sb[:, fb], wot_r[:, fb, :])

                # multiplicative causal masks for the 4 diagonal k-tiles:
                # cm[j][p, y] = 1.0 where j*128 + p <= y else 0.0
                cmasks = []
                if mode == "causal":
                    with tc.tile_pool(name="att_tmp", bufs=1) as tmppool:
                        for j in range(4):
                            m32 = tmppool.tile([128, 512], F32, tag="m32",
                                               name=f"m32_{j}")
                            nc.gpsimd.memset(m32[:], 1.0)
                            nc.gpsimd.affine_select(
                                out=m32[:], in_=m32[:],
                                compare_op=mybir.AluOpType.is_ge, fill=0.0,
                                base=-j * 128, pattern=[[1, 512]],
                                channel_multiplier=-1)
                            m16 = cpool.tile([128, 512], BF16, tag=f"cm{j}",
                                             name=f"cm{j}")
                            nc.vector.tensor_copy(m16[:], m32[:])
                            cmasks.append(m16)
                    # zero-init the p-tile slots once: diagonal units write
                    # only a column suffix and rely on finite stale data
                    for _ in range(5):
                        pz = ppool.tile([128, 512], BF16, tag="p")
                        nc.vector.memset(pz[:], 0.0)

                gdict = {}

                def prefetch_gather(qc, b):
                    # issue the gathered o_proj input load on the sync queue;
                    # emitted after a group's qkv loads so its AllGather wait
                    # does not delay them; transfer hides under attention
                    gath_r = gaths[(qc, b)].rearrange("g p ft t -> p g ft t")
                    g_sb = gpool.tile([128, c.n_cores, c.HL, 512], F16,
                                      tag="g")
                    nc.sync.dma_start(g_sb[:], gath_r)
                    gdict[(qc, b)] = g_sb

                def head_tail(qc, b, h, acc, pv_ps):
                    # denominators: all-reduce over partitions (k), then
                    # reciprocal; result broadcast on all partitions
                    accr = smpool.tile([128, 512], F32, tag="accr")
                    rq = smpool.tile([128, 512], F32, tag="rq")
                    nc.gpsimd.partition_all_reduce(
                        accr[:], acc[:], 128, bass_isa.ReduceOp.add)
                    nc.vector.reciprocal_approx_fast(rq[:], accr[:])
                    att_h = aopool.tile([128, 512], F16, tag="ao")
                    nc.vector.tensor_tensor(
                        att_h[:], pv_ps[:], rq[:], mybir.AluOpType.mult)
                    # stores ride the scalar queue (loads own the sync queue)
                    nc.scalar.dma_start(stages[(qc, b)][:, h, :], att_h[:])

                def emit_unit(qc, b, h, q_sb, k_sb, v_tok):
                    """scores+softmax+PV for one (head, 512-token q chunk)."""
                    nk = 4 * (qc + 1) if mode == "causal" else c.SQT
                    q0 = qc * 512
                    SKEW = 3
                    acc = smpool.tile([128, 512], F32, tag="acc")
                    pv_ps = ps_pv.tile([128, 512], F32, tag="pv")
                    p_tiles = {}
                    for kt in range(nk + SKEW):
                        if kt < nk:
                            # diagonal tiles: only columns >= j*128 are
                            # unmasked; the cmask multiply zeroes the rest
                            # (including whatever stale data sits there)
                            j = kt - (nk - 4) if mode == "causal" else -1
                            x0 = j * 128 if j > 0 else 0
                            s_ps = ps_s.tile([128, 512], F32, tag="s")
                            nc.tensor.matmul(
                                s_ps[:, x0:],
                                k_sb[:, kt * 128:(kt + 1) * 128],
                                q_sb[:, x0:],
                                start=True, stop=True)
                            if mode == "masked":
                                m_sb = mspool.tile([128, 512], F32, tag="m")
                                nc.sync.dma_start(
                                    m_sb[:],
                                    mask_ext[kt * 128:(kt + 1) * 128,
                                             q0:q0 + 512])
                                nc.vector.tensor_tensor(
                                    s_ps[:], s_ps[:], m_sb[:],
                                    mybir.AluOpType.add)
                            p_sb = ppool.tile([128, 512], BF16, tag="p")
                            nc.scalar.activation(
                                p_sb[:, x0:], s_ps[:, x0:],
                                mybir.ActivationFunctionType.Exp,
                                scale=inv_sqrt_dh)
                            if mode == "causal" and j >= 0:
                                nc.vector.tensor_tensor(
                                    p_sb[:], p_sb[:],
                                    cmasks[j][:],
                                    mybir.AluOpType.mult)
                            if kt == 0:
                                nc.vector.tensor_copy(acc[:], p_sb[:])
                            else:
                                nc.vector.tensor_tensor(
                                    acc[:], acc[:], p_sb[:],
                                    mybir.AluOpType.add)
                            p_tiles[kt] = p_sb
                        if kt >= SKEW:
                            j = kt - SKEW
                            nc.tensor.matmul(
                                pv_ps[:], v_tok[:, j, :], p_tiles[j][:],
                                start=(j == 0), stop=(j == nk - 1))
                            del p_tiles[j]
                    return acc, pv_ps

                def emit_oproj(qc, b):
                    # out[j, t] for this core's 640 hidden cols, 512 tokens
                    g_sb = gdict.pop((qc, b))
                    tg0 = b * c.S + qc * 512
                    for jb in range(c.JB):
                        cps = ps_c.tile([128, 512], F32, tag="cps")
                        for f in range(c.KT):
                            nc.tensor.matmul(
                                cps[:],
                                wo_sb[:, f, jb * 128:(jb + 1) * 128],
                                g_sb[:, f // c.HL, f % c.HL, :],
                                start=(f == 0), stop=(f == c.KT - 1))
                        co = copool.tile([128, 512], F16, tag="co")
                        nc.vector.tensor_copy(co[:], cps[:])
                        nc.scalar.dma_start(
                            out_ext[jb * 128:(jb + 1) * 128, tg0:tg0 + 512],
                            co[:])

                def emit_loads(qcs, b, h, nk_max):
                    t0 = b * c.S
                    # alternate HWDGE queues per head: a group's load burst
                    # (up to ~6MB) would otherwise serialize on one queue
                    eng = nc.sync if h % 2 == 0 else nc.scalar
                    qs = {}
                    for qc in qcs:
                        q_sb = qpool.tile([128, 512], F16, tag="q")
                        eng.dma_start(
                            q_sb[:],
                            qkt[h * 128:(h + 1) * 128,
                                t0 + qc * 512:t0 + qc * 512 + 512])
                        qs[qc] = q_sb
                    k_sb = kpool.tile([128, c.S], F16, tag="k")
                    eng.dma_start(
                        k_sb[:, :nk_max * 128],
                        qkt[(c.HL + h) * 128:(c.HL + h + 1) * 128,
                            t0:t0 + nk_max * 128])
                    v_tok = vpool.tile([128, c.SQT, 128], BF16, tag="v")
                    nc.sync.dma_start(
                        v_tok[:, :nk_max, :],
                        vt[h * 128:(h + 1) * 128, t0:t0 + nk_max * 128],
                        transpose=True)
                    return qs, k_sb, v_tok

                def emit_ag(qc, b):
                    nc.gpsimd.collective_compute(
                        "AllGather",
                        mybir.AluOpType.bypass,
                        replica_groups=[list(range(c.n_cores))],
                        ins=[stages[(qc, b)][:].opt()],
                        outs=[gaths[(qc, b)][:].opt()],
                    )

                def emit_group(qcs, b, c_blocks, own_qa_c=False):
                    """attention for all heads x q-chunks in qcs for batch b;
                    k/v loaded once per head; c_blocks (ready earlier blocks)
                    interleave their o_proj into this group.  qc-major unit
                    order: each chunk's AllGather fires as early as possible
                    (a full group before its o_proj consumes it).  With
                    own_qa_c, the group's own first chunk is o_proj'ed at its
                    end (its AllGather fires mid-group)."""
                    if mode == "causal":
                        nk_max = 4 * (max(qcs) + 1)
                    else:
                        nk_max = c.SQT
                    loads = {}
                    for h in range(min(2, c.HL)):
                        loads[h] = emit_loads(qcs, b, h, nk_max)
                    for cb in c_blocks:
                        prefetch_gather(*cb)
                    units = [(h, qc) for qc in qcs for h in range(c.HL)]
                    nu = len(units)
                    cpos = {}
                    if c_blocks:
                        step = max(1, (nu - 2) // (len(c_blocks) + 1))
                        for ci, cb in enumerate(c_blocks):
                            cpos[min((ci + 1) * step, nu - 2)] = cb
                    ppos = {}
                    if own_qa_c:
                        ppos[nu - 4] = (qcs[0], b)
                        cpos[nu - 1] = (qcs[0], b)

                    def pop_tail():
                        tqc, tb, th, tacc, tpv = tails.pop()
                        head_tail(tqc, tb, th, tacc, tpv)
                        if th == c.HL - 1:
                            emit_ag(tqc, tb)

                    tails = []
                    for ui, (h, qc) in enumerate(units):
                        q_sb = loads[h][0][qc]
                        k_sb = loads[h][1]
                        v_tok = loads[h][2]
                        acc, pv_ps = emit_unit(qc, b, h, q_sb, k_sb, v_tok)
                        if qc == qcs[0] and h + 2 < c.HL:
                            loads[h + 2] = emit_loads(qcs, b, h + 2, nk_max)
                        # skew the normalize tail by one unit so the gpsimd
                        # all-reduce latency hides under the next unit
                        if tails:
                            pop_tail()
                        tails.append((qc, b, h, acc, pv_ps))
                        if ui in ppos:
                            prefetch_gather(*ppos[ui])
                        if ui in cpos:
                            emit_oproj(*cpos[ui])
                    while tails:
                        pop_tail()

                sb0, sb1 = [0, 1], [2, 3]
                emit_group(sb0, 0, [])
                emit_group(sb0, 1, [(0, 0), (1, 0)])
                emit_group(sb1, 0, [(0, 1), (1, 1)])
                emit_group(sb1, 1, [(2, 0), (3, 0)], own_qa_c=True)
                prefetch_gather(3, 1)
                emit_oproj(3, 1)
    nc.compile()
    return nc


# --------------------------------------------------------------------------
_CACHE = {}


def _get_program(cfg: Cfg, mode: str):
    key = (cfg.key(), mode)
    if key not in _CACHE:
        _CACHE[key] = build_program(cfg, mode)
    return _CACHE[key]


def prepare_inputs(cfg: Cfg, hidden_states, attention_mask, W_pack, W_o):
    """Host-side shard + layout prep. Returns (mode, in_maps)."""
    c = cfg
    X = np.asarray(hidden_states, dtype=np.float32).reshape(c.T, c.hidden)
    XT = np.ascontiguousarray(X.T.astype(np.float16))

    mask = np.asarray(attention_mask, dtype=np.float32).reshape(c.S, c.S)
    causal_ref = np.where(
        np.tril(np.ones((c.S, c.S), dtype=bool)), 0.0, -1e9
    ).astype(np.float32)
    if np.array_equal(mask, causal_ref):
        mode = "causal"
    elif not mask.any():
        mode = "dense"
    else:
        mode = "masked"

    W_pack = np.asarray(W_pack, dtype=np.float32)
    W_o = np.asarray(W_o, dtype=np.float32)
    H = c.hidden
    in_maps = []
    for g in range(c.n_cores):
        r0, r1 = g * c.FO, (g + 1) * c.FO
        wq = W_pack[r0:r1]
        wk = W_pack[H + r0:H + r1]
        wv = W_pack[2 * H + r0:2 * H + r1]
        wqkvT = np.ascontiguousarray(
            np.concatenate([wq, wk, wv], axis=0).T.astype(np.float16))
        woT = np.ascontiguousarray(W_o[r0:r1, :].T.astype(np.float16))
        m = {"xt": XT, "wqkvt": wqkvT, "wot": woT}
        if mode == "masked":
            m["maskt"] = np.ascontiguousarray((mask * math.sqrt(c.dh)).T)
        in_maps.append(m)
    return mode, in_maps


def assemble_output(cfg: Cfg, results):
    c = cfg
    full = np.empty((c.T, c.hidden), dtype=np.float32)
    for g in range(c.n_cores):
        o = results[g]["out"]  # [FO, T] fp16
        full[:, g * c.FO:(g + 1) * c.FO] = o.astype(np.float32).T
    return full.reshape(c.B, c.S, c.hidden)


def kernel(hidden_states, attention_mask, W_pack, W_o):
    cfg = Cfg()
    mode, in_maps = prepare_inputs(cfg, hidden_states, attention_mask,
                                   W_pack, W_o)
    nc = _get_program(cfg, mode)
    res = bass_utils.run_bass_kernel_spmd(nc, in_maps,
                                          list(range(cfg.n_cores)))
    return assemble_output(cfg, res.results)


# revision 48
# speedup vs baseline: 1.0240x; 1.0240x over previous
"""Trainium2 Bass kernel for BaichuanAttention (hidden=5120, 40 heads, b=2, s=2048).

Tensor-parallel over heads across 8 NeuronCores, all matmuls in fp16
(full PE rate, fp32 PSUM accumulation):

  Phase A: per-core QKV projection (sharded W_pack rows), output
           feature-major qkvt in DRAM (fp16).
  Phase B: causal attention with transposed-scores formulation:
           S^T[k,q] = K_tile^T @ Q directly gives the P^T layout the PV
           matmul needs -- no per-tile transposes.  exp on the scalar
           engine (constant bias keeps P in fp16 range), row-sums via
           vector adds + gpsimd partition_all_reduce, V loaded
           token-major via DMA transpose.
  Phase C (fused into B's q-chunk loop): AllGather each finished
           attnT chunk across cores, then each core computes o_proj for
           its 640 hidden columns (full 5120-feature contraction) -- no
           ReduceScatter on the critical path; output is column-sharded.
"""

import math
import sys
from collections import deque

for _p in ("/opt/trn_rl_repo",):
    if _p not in sys.path:
        sys.path.insert(0, _p)

import numpy as np

import concourse.bass as bass
import concourse.bass_isa as bass_isa
import concourse.mybir as mybir
import concourse.tile as tile
from concourse import bacc, bass_utils

F16 = mybir.dt.float16
BF16 = mybir.dt.bfloat16
F32 = mybir.dt.float32


class Cfg:
    def __init__(self, hidden=5120, n_heads=40, dh=128, B=2, S=2048, n_cores=8):
        self.hidden = hidden
        self.n_heads = n_heads
        self.dh = dh
        self.B = B
        self.S = S
        self.n_cores = n_cores
        assert dh == 128
        self.HL = n_heads // n_cores          # heads per core (5)
        self.F = 3 * self.HL * dh             # per-core packed qkv rows (1920)
        self.FO = self.HL * dh                # per-core attn feature width (640)
        self.T = B * S                        # total tokens (4096)
        self.KT = hidden // 128               # contraction tiles for qkv (40)
        self.TC = self.T // 512               # token chunks for qkv (8)
        self.QC = S // 512                    # q chunks per batch (4)
        self.SQT = S // 128                   # k tiles per batch (16)
        self.JB = self.FO // 128              # out column blocks per core (5)

    def key(self):
        return (self.hidden, self.n_heads, self.dh, self.B, self.S, self.n_cores)


def build_program(cfg: Cfg, mode: str):
    """mode: 'causal' (causal skip + multiplicative tri masks),
    'dense' (no mask), 'masked' (additive mask input, pre-scaled and
    pre-transposed on host)."""
    assert mode in ("causal", "dense", "masked")
    c = cfg
    nc = bacc.Bacc("TRN2", target_bir_lowering=False, debug=False,
                   num_devices=c.n_cores)
    xt = nc.dram_tensor("xt", [c.hidden, c.T], F16, kind="ExternalInput").ap()
    wqkvt = nc.dram_tensor("wqkvt", [c.hidden, c.F], F16,
                           kind="ExternalInput").ap()
    wot = nc.dram_tensor("wot", [c.hidden, c.FO], F16,
                         kind="ExternalInput").ap()
    mask_ext = None
    if mode == "masked":
        # maskT[k, q] = mask[q, k] * sqrt(dh), fp32
        mask_ext = nc.dram_tensor("maskt", [c.S, c.S], F32,
                                  kind="ExternalInput").ap()
    # column-sharded transposed output: rows = this core's 640 hidden cols
    out_ext = nc.dram_tensor("out", [c.FO, c.T], F16,
                             kind="ExternalOutput").ap()

    inv_sqrt_dh = 1.0 / math.sqrt(c.dh)

    with tile.TileContext(nc) as tc:
        with tc.tile_pool(name="dram", bufs=1, space="DRAM") as dram:
            # q,k features in fp16; v features in bf16 (the softmax P tiles
            # must be bf16 for range, and the PV matmul needs matching dtypes)
            qkt = dram.tile([2 * c.FO, c.T], F16, tag="qkt", name="qkt")
            vt = dram.tile([c.FO, c.T], BF16, tag="vt", name="vt")
            # p-major stage layout: the gathered o_proj input then reads as
            # 5KB-contiguous runs per partition (big DMA descriptors)
            stages = {}
            gaths = {}
            for qc in range(c.QC):
                for b in range(c.B):
                    stages[(qc, b)] = dram.tile(
                        [128, c.HL, 512], F16, tag=f"st{qc}_{b}",
                        name=f"st{qc}_{b}")
                    gaths[(qc, b)] = dram.tile(
                        [c.n_cores, 128, c.HL, 512], F16, tag=f"g{qc}_{b}",
                        name=f"g{qc}_{b}", addr_space="Shared")

            # ---------------- Phase A: QKV projection -------------------
            # qkvt[f, t] = sum_h wqkvt[h, f] * xt[h, t]
            wq_r = wqkvt.rearrange("(ko p) f -> p ko f", p=128)
            xt_r = xt.rearrange("(ko p) t -> p ko t", p=128)
            qkt_r = qkt.rearrange("(ft p) t -> ft p t", p=128)
            vt_r = vt.rearrange("(ft p) t -> ft p t", p=128)
            n_qk_ft = 2 * c.FO // 128  # 10
            splits = [8, 7]
            assert sum(splits) == c.F // 128
            with tc.tile_pool(name="qkv_w", bufs=2) as wpool, \
                 tc.tile_pool(name="qkv_x", bufs=20) as xpool, \
                 tc.tile_pool(name="qkv_o", bufs=8) as opool, \
                 tc.tile_pool(name="qkv_ps", bufs=8, space="PSUM") as pspool:
                ft0 = 0
                for nft in splits:
                    w_sb = wpool.tile([128, c.KT, nft * 128], F16, tag="w")
                    for kq in range(c.KT):
                        nc.sync.dma_start(
                            w_sb[:, kq],
                            wq_r[:, kq, ft0 * 128:(ft0 + nft) * 128])
                    # order 0,4,...: batch-0 and batch-1 early-token chunks
                    # finish first so Phase B can start during A's last pass
                    for tci in (0, 4, 1, 5, 2, 6, 3, 7):
                        pss = [pspool.tile([128, 512], F32, tag="ps",
                                           name=f"ps{i}")
                               for i in range(nft)]
                        for k in range(c.KT):
                            x_sb = xpool.tile([128, 512], F16, tag="x")
                            # alternate queues: 84MB of x needs both HWDGE
                            # queues to stay ahead of the PE
                            eng = nc.scalar if k % 2 == 0 else nc.sync
                            eng.dma_start(
                                x_sb[:], xt_r[:, k, tci * 512:(tci + 1) * 512])
                            for i in range(nft):
                                nc.tensor.matmul(
                                    pss[i][:],
                                    w_sb[:, k, i * 128:(i + 1) * 128],
                                    x_sb[:],
                                    start=(k == 0), stop=(k == c.KT - 1))
                        for i in range(nft):
                            ft = ft0 + i
                            if ft < n_qk_ft:
                                o_sb = opool.tile([128, 512], F16, tag="o")
                                dst = qkt_r[ft, :, tci * 512:(tci + 1) * 512]
                            else:
                                o_sb = opool.tile([128, 512], BF16, tag="ov")
                                dst = vt_r[ft - n_qk_ft, :,
                                           tci * 512:(tci + 1) * 512]
                            nc.vector.tensor_copy(o_sb[:], pss[i][:])
                            nc.scalar.dma_start(dst, o_sb[:])
                    ft0 += nft

            # ------------- Phase B + C: attention + o_proj --------------
            wot_r = wot.rearrange("(fb p) j -> p fb j", p=128)
            with tc.tile_pool(name="att_c", bufs=1) as cpool, \
                 tc.tile_pool(name="att_q", bufs=10) as qpool, \
                 tc.tile_pool(name="att_k", bufs=5) as kpool, \
                 tc.tile_pool(name="att_v", bufs=5) as vpool, \
                 tc.tile_pool(name="att_p", bufs=5) as ppool, \
                 tc.tile_pool(name="att_sm", bufs=2) as smpool, \
                 tc.tile_pool(name="att_o", bufs=2) as aopool, \
                 tc.tile_pool(name="att_ms", bufs=(4 if mode == "masked" else 1)) as mspool, \
                 tc.tile_pool(name="op_w", bufs=1) as wopool, \
                 tc.tile_pool(name="op_g", bufs=2) as gpool, \
                 tc.tile_pool(name="op_o", bufs=2) as copool, \
                 tc.tile_pool(name="ps_s", bufs=3, space="PSUM") as ps_s, \
                 tc.tile_pool(name="ps_pv", bufs=2, space="PSUM") as ps_pv, \
                 tc.tile_pool(name="ps_c", bufs=2, space="PSUM") as ps_c:

                # resident W_o slice: [128, 40, 640] fp16 (~51KB/partition)
                wo_sb = wopool.tile([128, c.KT, c.FO], F16)
                for fb in range(c.KT):
                    nc.sync.dma_start(wo_sb[:, fb], wot_r[:, fb, :])

                # multiplicative causal masks for the 4 diagonal k-tiles:
                # cm[j][p, y] = 1.0 where j*128 + p <= y else 0.0
                cmasks = []
                if mode == "causal":
                    with tc.tile_pool(name="att_tmp", bufs=1) as tmppool:
                        for j in range(4):
                            m32 = tmppool.tile([128, 512], F32, tag="m32",
                                               name=f"m32_{j}")
                            nc.gpsimd.memset(m32[:], 1.0)
                            nc.gpsimd.affine_select(
                                out=m32[:], in_=m32[:],
                                compare_op=mybir.AluOpType.is_ge, fill=0.0,
                                base=-j * 128, pattern=[[1, 512]],
                                channel_multiplier=-1)
                            m16 = cpool.tile([128, 512], BF16, tag=f"cm{j}",
                                             name=f"cm{j}")
                            nc.vector.tensor_copy(m16[:], m32[:])
                            cmasks.append(m16)
                    # zero-init the p-tile slots once: diagonal units write
                    # only a column suffix and rely on finite stale data
                    for _ in range(5):
                        pz = ppool.tile([128, 512], BF16, tag="p")
                        nc.vector.memset(pz[:], 0.0)

                gdict = {}

                def prefetch_gather(qc, b):
                    # issue the gathered o_proj input load on the sync queue;
                    # emitted after a group's qkv loads so its AllGather wait
                    # does not delay them; transfer hides under attention
                    gath_r = gaths[(qc, b)].rearrange("g p ft t -> p g ft t")
                    g_sb = gpool.tile([128, c.n_cores, c.HL, 512], F16,
                                      tag="g")
                    nc.sync.dma_start(g_sb[:], gath_r)
                    gdict[(qc, b)] = g_sb

                def head_tail(qc, b, h, acc, pv_ps):
                    # denominators: all-reduce over partitions (k), then
                    # reciprocal; result broadcast on all partitions
                    accr = smpool.tile([128, 512], F32, tag="accr")
                    rq = smpool.tile([128, 512], F32, tag="rq")
                    nc.gpsimd.partition_all_reduce(
                        accr[:], acc[:], 128, bass_isa.ReduceOp.add)
                    nc.vector.reciprocal_approx_fast(rq[:], accr[:])
                    att_h = aopool.tile([128, 512], F16, tag="ao")
                    nc.vector.tensor_tensor(
                        att_h[:], pv_ps[:], rq[:], mybir.AluOpType.mult)
                    # stores ride the scalar queue (loads own the sync queue)
                    nc.scalar.dma_start(stages[(qc, b)][:, h, :], att_h[:])

                def emit_unit(qc, b, h, q_sb, k_sb, v_tok):
                    """scores+softmax+PV for one (head, 512-token q chunk)."""
                    nk = 4 * (qc + 1) if mode == "causal" else c.SQT
                    q0 = qc * 512
                    SKEW = 3
                    acc = smpool.tile([128, 512], F32, tag="acc")
                    pv_ps = ps_pv.tile([128, 512], F32, tag="pv")
                    p_tiles = {}
                    for kt in range(nk + SKEW):
                        if kt < nk:
                            # diagonal tiles: only columns >= j*128 are
                            # unmasked; the cmask multiply zeroes the rest
                            # (including whatever stale data sits there)
                            j = kt - (nk - 4) if mode == "causal" else -1
                            x0 = j * 128 if j > 0 else 0
                            s_ps = ps_s.tile([128, 512], F32, tag="s")
                            nc.tensor.matmul(
                                s_ps[:, x0:],
                                k_sb[:, kt * 128:(kt + 1) * 128],
                                q_sb[:, x0:],
                                start=True, stop=True)
                            if mode == "masked":
                                m_sb = mspool.tile([128, 512], F32, tag="m")
                                nc.sync.dma_start(
                                    m_sb[:],
                                    mask_ext[kt * 128:(kt + 1) * 128,
                                             q0:q0 + 512])
                                nc.vector.tensor_tensor(
                                    s_ps[:], s_ps[:], m_sb[:],
                                    mybir.AluOpType.add)
                            p_sb = ppool.tile([128, 512], BF16, tag="p")
                            nc.scalar.activation(
                                p_sb[:, x0:], s_ps[:, x0:],
                                mybir.ActivationFunctionType.Exp,
                                scale=inv_sqrt_dh)
                            if mode == "causal" and j >= 0:
                                nc.vector.tensor_tensor(
                                    p_sb[:], p_sb[:],
                                    cmasks[j][:],
                                    mybir.AluOpType.mult)
                            if kt == 0:
                                nc.vector.tensor_copy(acc[:], p_sb[:])
                            else:
                                nc.vector.tensor_tensor(
                                    acc[:], acc[:], p_sb[:],
                                    mybir.AluOpType.add)
                            p_tiles[kt] = p_sb
                        if kt >= SKEW:
                            j = kt - SKEW
                            nc.tensor.matmul(
                                pv_ps[:], v_tok[:, j, :], p_tiles[j][:],
                                start=(j == 0), stop=(j == nk - 1))
                            del p_tiles[j]
                    return acc, pv_ps

                def emit_oproj(qc, b):
                    # out[j, t] for this core's 640 hidden cols, 512 tokens
                    g_sb = gdict.pop((qc, b))
                    tg0 = b * c.S + qc * 512
                    for jb in range(c.JB):
                        cps = ps_c.tile([128, 512], F32, tag="cps")
                        for f in range(c.KT):
                            nc.tensor.matmul(
                                cps[:],
                                wo_sb[:, f, jb * 128:(jb + 1) * 128],
                                g_sb[:, f // c.HL, f % c.HL, :],
                                start=(f == 0), stop=(f == c.KT - 1))
                        co = copool.tile([128, 512], F16, tag="co")
                        nc.vector.tensor_copy(co[:], cps[:])
                        nc.scalar.dma_start(
                            out_ext[jb * 128:(jb + 1) * 128, tg0:tg0 + 512],
                            co[:])

                def emit_loads(qcs, b, h, nk_max):
                    t0 = b * c.S
                    # all attention loads on sync: the scalar queue's exp
                    # stream delays DMA issues placed behind it
                    eng = nc.sync
                    qs = {}
                    for qc in qcs:
                        q_sb = qpool.tile([128, 512], F16, tag="q")
                        eng.dma_start(
                            q_sb[:],
                            qkt[h * 128:(h + 1) * 128,
                                t0 + qc * 512:t0 + qc * 512 + 512])
                        qs[qc] = q_sb
                    k_sb = kpool.tile([128, c.S], F16, tag="k")
                    eng.dma_start(
                        k_sb[:, :nk_max * 128],
                        qkt[(c.HL + h) * 128:(c.HL + h + 1) * 128,
                            t0:t0 + nk_max * 128])
                    v_tok = vpool.tile([128, c.SQT, 128], BF16, tag="v")
                    nc.sync.dma_start(
                        v_tok[:, :nk_max, :],
                        vt[h * 128:(h + 1) * 128, t0:t0 + nk_max * 128],
                        transpose=True)
                    return qs, k_sb, v_tok

                def emit_ag(qc, b):
                    nc.gpsimd.collective_compute(
                        "AllGather",
                        mybir.AluOpType.bypass,
                        replica_groups=[list(range(c.n_cores))],
                        ins=[stages[(qc, b)][:].opt()],
                        outs=[gaths[(qc, b)][:].opt()],
                    )

                def emit_group(qcs, b, c_blocks, own_qa_c=False):
                    """attention for all heads x q-chunks in qcs for batch b;
                    k/v loaded once per head; c_blocks (ready earlier blocks)
                    interleave their o_proj into this group.  qc-major unit
                    order: each chunk's AllGather fires as early as possible
                    (a full group before its o_proj consumes it).  With
                    own_qa_c, the group's own first chunk is o_proj'ed at its
                    end (its AllGather fires mid-group)."""
                    if mode == "causal":
                        nk_max = 4 * (max(qcs) + 1)
                    else:
                        nk_max = c.SQT
                    loads = {}
                    for h in range(min(2, c.HL)):
                        loads[h] = emit_loads(qcs, b, h, nk_max)
                    for cb in c_blocks:
                        prefetch_gather(*cb)
                    units = [(h, qc) for qc in qcs for h in range(c.HL)]
                    nu = len(units)
                    cpos = {}
                    if c_blocks:
                        step = max(1, (nu - 2) // (len(c_blocks) + 1))
                        for ci, cb in enumerate(c_blocks):
                            cpos[min((ci + 1) * step, nu - 2)] = cb
                    ppos = {}
                    if own_qa_c:
                        ppos[nu - 4] = (qcs[0], b)
                        cpos[nu - 1] = (qcs[0], b)

                    def pop_tail():
                        tqc, tb, th, tacc, tpv = tails.pop()
                        head_tail(tqc, tb, th, tacc, tpv)
                        if th == c.HL - 1:
                            emit_ag(tqc, tb)

                    tails = []
                    for ui, (h, qc) in enumerate(units):
                        q_sb = loads[h][0][qc]
                        k_sb = loads[h][1]
                        v_tok = loads[h][2]
                        acc, pv_ps = emit_unit(qc, b, h, q_sb, k_sb, v_tok)
                        if qc == qcs[0] and h + 2 < c.HL:
                            loads[h + 2] = emit_loads(qcs, b, h + 2, nk_max)
                        # skew the normalize tail by one unit so the gpsimd
                        # all-reduce latency hides under the next unit
                        if tails:
                            pop_tail()
                        tails.append((qc, b, h, acc, pv_ps))
                        if ui in ppos:
                            prefetch_gather(*ppos[ui])
                        if ui in cpos:
                            emit_oproj(*cpos[ui])
                    while tails:
                        pop_tail()

                sb0, sb1 = [0, 1], [2, 3]
                emit_group(sb0, 0, [])
                emit_group(sb0, 1, [(0, 0), (1, 0)])
                emit_group(sb1, 0, [(0, 1), (1, 1)])
                emit_group(sb1, 1, [(2, 0), (3, 0)], own_qa_c=True)
                prefetch_gather(3, 1)
                emit_oproj(3, 1)
    nc.compile()
    return nc


# --------------------------------------------------------------------------
_CACHE = {}


def _get_program(cfg: Cfg, mode: str):
    key = (cfg.key(), mode)
    if key not in _CACHE:
        _CACHE[key] = build_program(cfg, mode)
    return _CACHE[key]


def prepare_inputs(cfg: Cfg, hidden_states, attention_mask, W_pack, W_o):
    """Host-side shard + layout prep. Returns (mode, in_maps)."""
    c = cfg
    X = np.asarray(hidden_states, dtype=np.float32).reshape(c.T, c.hidden)
    XT = np.ascontiguousarray(X.T.astype(np.float16))

    mask = np.asarray(attention_mask, dtype=np.float32).reshape(c.S, c.S)
    causal_ref = np.where(
        np.tril(np.ones((c.S, c.S), dtype=bool)), 0.0, -1e9
    ).astype(np.float32)
    if np.array_equal(mask, causal_ref):
        mode = "causal"
    elif not mask.any():
        mode = "dense"
    else:
        mode = "masked"

    W_pack = np.asarray(W_pack, dtype=np.float32)
    W_o = np.asarray(W_o, dtype=np.float32)
    H = c.hidden
    in_maps = []
    for g in range(c.n_cores):
        r0, r1 = g * c.FO, (g + 1) * c.FO
        wq = W_pack[r0:r1]
        wk = W_pack[H + r0:H + r1]
        wv = W_pack[2 * H + r0:2 * H + r1]
        wqkvT = np.ascontiguousarray(
            np.concatenate([wq, wk, wv], axis=0).T.astype(np.float16))
        woT = np.ascontiguousarray(W_o[r0:r1, :].T.astype(np.float16))
        m = {"xt": XT, "wqkvt": wqkvT, "wot": woT}
        if mode == "masked":
            m["maskt"] = np.ascontiguousarray((mask * math.sqrt(c.dh)).T)
        in_maps.append(m)
    return mode, in_maps


def assemble_output(cfg: Cfg, results):
    c = cfg
    full = np.empty((c.T, c.hidden), dtype=np.float32)
    for g in range(c.n_cores):
        o = results[g]["out"]  # [FO, T] fp16
        full[:, g * c.FO:(g + 1) * c.FO] = o.astype(np.float32).T
    return full.reshape(c.B, c.S, c.hidden)


def kernel(hidden_states, attention_mask, W_pack, W_o):
    cfg = Cfg()
    mode, in_maps = prepare_inputs(cfg, hidden_states, attention_mask,
                                   W_pack, W_o)
    nc = _get_program(cfg, mode)
    res = bass_utils.run_bass_kernel_spmd(nc, in_maps,
                                          list(range(cfg.n_cores)))
    return assemble_output(cfg, res.results)
